# revision 41
# baseline (speedup 1.0000x reference)
"""Trainium2 Bass kernel for packed-varlen causal attention (16 heads, D=1024).

Strategy: data-parallel over segments across 8 NeuronCores. Each core packs
1-2 segments tile-aligned into a 1536-token buffer. One SPMD program; all
per-core differences are data (packed inputs + mask tiles).

v2 design notes (driven by the TRN2 cost model):
- PE work minimized: matmul cost is (moving free rows) x pe_cycle, so all
  matmuls use 512-wide moving operands; biases are folded off the PE (q-bias
  fused into RoPE DVE ops, v/out biases folded into a host-side constant).
- PE stream kept dense (QK x18 then PV x18 per head-pair, 4 PSUM banks for
  PV accumulators) so the PE p-state stays at 2.4 GHz.
- Mask multiplies only where the union block structure actually needs them
  (computed at plan time), and they run on the otherwise-idle Pool engine.
- Softmax normalization: denominators via the ones-column-in-V trick, then
  reciprocal_approx_fast (not the 6x slower InstReciprocal), broadcast via a
  tiny contract-2 PE matmul, applied by DVE mul straight out of PSUM.
- bf16 activations everywhere off the PSUM accumulators.
"""
import os
from contextlib import ExitStack

import numpy as np
import ml_dtypes

import concourse.bass as bass
import concourse.tile as tile
from concourse import bacc, mybir
from concourse.bass_utils import run_bass_kernel_spmd

BF16 = ml_dtypes.bfloat16
F8 = ml_dtypes.float8_e4m3fn
F32 = np.float32
NCORES = 8
NT = 12            # token tiles of 128 -> 1536 token slots per core
TOK = NT * 128
EMBED, HEADS, HDIM = 1024, 16, 64
NQC = 3            # 512-wide q chunks
DT = mybir.dt

LAST_EXEC_NS = None
LAST_TRACE = None
_CACHE = {}


def _install_ntff_shim():
    """Provide antenv.axon_hooks (missing in this image) so
    run_bass_kernel_spmd(trace=True) can capture NTFF profiles via the
    axon .so, and keep artifacts local instead of uploading."""
    import sys
    import types
    try:
        import antenv.axon_hooks  # noqa: F401
        return
    except ImportError:
        pass
    try:
        from trn_agent_boot.trn_boot import _ntff_profile_via_ctypes
        hook = _ntff_profile_via_ctypes("/opt/axon/libaxon_pjrt.so")
    except Exception:
        hook = None
    mod = types.ModuleType("antenv.axon_hooks")
    mod.get_axon_ntff_profile_hook = lambda: hook
    mod.set_axon_ntff_profile_hook = lambda h: None
    sys.modules["antenv.axon_hooks"] = mod
    import concourse.bass_utils as _bu
    _bu.upload_artifacts = lambda tmpdir: tmpdir


# ---------------------------------------------------------------- planning --

def _core_token_maps(core_chunks):
    """Per-core segid/pos/valid arrays over the TOK slots."""
    maps = []
    for chunks in core_chunks:
        segid = np.full(TOK, -1, np.int64)
        pos = np.zeros(TOK, np.int64)
        for (s, t0, L) in chunks:
            sl = slice(t0 * 128, t0 * 128 + L)
            segid[sl] = s
            pos[sl] = np.arange(L)
        maps.append((segid, pos))
    return maps


def _build_plan(seq_lens):
    segs = sorted(range(len(seq_lens)), key=lambda i: -int(seq_lens[i]))
    loads = [0.0] * NCORES
    tiles_used = [0] * NCORES
    assign = [[] for _ in range(NCORES)]
    for s in segs:
        L = int(seq_lens[s])
        nt = (L + 127) // 128
        cost = L * 8.4e6 + (L * L) * 2048.0
        placed = False
        for c in sorted(range(NCORES), key=lambda c: loads[c]):
            if tiles_used[c] + nt <= NT:
                assign[c].append(s)
                loads[c] += cost
                tiles_used[c] += nt
                placed = True
                break
        assert placed, "segments do not fit the 8x1536 structure"
    core_chunks = []
    for c in range(NCORES):
        t0, chunks = 0, []
        for s in assign[c]:
            L = int(seq_lens[s])
            chunks.append((s, t0, L))
            t0 += (L + 127) // 128
        core_chunks.append(chunks)

    # union of per-core block-causal tile pairs (kj, qi)
    pairs = set()
    for chunks in core_chunks:
        for (_, t0, L) in chunks:
            nt = (L + 127) // 128
            for a in range(nt):
                for b in range(a + 1):
                    pairs.add((t0 + b, t0 + a))  # (kj, qi), kj <= qi

    # which pairs need a mask multiply: on some core, a valid q column in qi
    # has a key in kj that is not (same segment & causal & valid)
    maps = _core_token_maps(core_chunks)
    def pair_needs_mask(kj, qi):
        for segid, pos in maps:
            ks = slice(kj * 128, kj * 128 + 128)
            qs = slice(qi * 128, qi * 128 + 128)
            segk, posk = segid[ks], pos[ks]
            segq, posq = segid[qs], pos[qs]
            validq = segq >= 0
            if not validq.any():
                continue
            ok = ((segk[:, None] == segq[None, :]) & (segk[:, None] >= 0)
                  & (posk[:, None] <= posq[None, :]))
            # columns with any not-ok key, restricted to valid q columns
            bad = (~ok).any(axis=0) & validq
            if bad.any():
                return True
        return False

    maskset = {p for p in pairs if pair_needs_mask(*p)}

    # per q-chunk (cch) key lists with contiguous q ranges and mask runs
    qhi_u = {}
    for (kj, qi) in pairs:
        qhi_u[kj] = max(qhi_u.get(kj, -1), qi)
    for kj in qhi_u:  # sanity: contiguity of qi range [kj, qhi_u]
        for qi in range(kj, qhi_u[kj] + 1):
            assert (kj, qi) in pairs, "non-contiguous union structure"

    structure = []   # [cch] -> list of (kj, qlo_t, qhi_t, [(qa,qb,moff)..])
    moff = 0
    for cch in range(NQC):
        klist = []
        for kj in sorted(qhi_u):
            qlo_t = max(kj, 4 * cch)
            qhi_t = min(qhi_u[kj], 4 * cch + 3)
            if qlo_t > qhi_t:
                continue
            runs = []
            qa = None
            for qi in range(qlo_t, qhi_t + 1):
                if (kj, qi) in maskset:
                    if qa is None:
                        qa = qi
                else:
                    if qa is not None:
                        runs.append((qa, qi, moff))
                        moff += (qi - qa) * 128
                        qa = None
            if qa is not None:
                runs.append((qa, qhi_t + 1, moff))
                moff += (qhi_t + 1 - qa) * 128
            klist.append((kj, qlo_t, qhi_t, tuple(runs)))
        structure.append(tuple(klist))
    mask_cols = max(moff, 128)
    return core_chunks, tuple(structure), mask_cols


# ---------------------------------------------------------- device program --

def _emit_program(structure, mask_cols):
    nc = bacc.Bacc("TRN2", target_bir_lowering=False, debug=False,
                   num_devices=NCORES)
    f32, bf16 = DT.float32, DT.bfloat16
    EXP = mybir.ActivationFunctionType.Exp
    ADD = mybir.AluOpType.add
    MUL = mybir.AluOpType.mult

    xT_d = nc.dram_tensor("xT", [EMBED, TOK], bf16, kind="ExternalInput").ap()
    cosT_d = nc.dram_tensor("cosT", [128, TOK], bf16, kind="ExternalInput").ap()
    sinT_d = nc.dram_tensor("sinT", [128, TOK], bf16, kind="ExternalInput").ap()
    mask_d = nc.dram_tensor("maskb", [128, mask_cols], bf16,
                            kind="ExternalInput").ap()
    wq_d = nc.dram_tensor("wqT", [EMBED, EMBED], bf16, kind="ExternalInput").ap()
    wk_d = nc.dram_tensor("wkT", [EMBED, EMBED], bf16, kind="ExternalInput").ap()
    wv_d = nc.dram_tensor("wvT", [EMBED, EMBED], bf16, kind="ExternalInput").ap()
    wo_d = nc.dram_tensor("woT", [EMBED, EMBED], bf16, kind="ExternalInput").ap()
    qb_d = nc.dram_tensor("qb", [128, 8], f32, kind="ExternalInput").ap()
    qbr_d = nc.dram_tensor("qbr", [128, 8], f32, kind="ExternalInput").ap()
    sel_d = nc.dram_tensor("sel2", [2, 128], DT.float32r,
                           kind="ExternalInput").ap()
    perm_d = nc.dram_tensor("permM", [128, 128], bf16, kind="ExternalInput").ap()
    yT_d = nc.dram_tensor("yT", [EMBED, TOK], bf16, kind="ExternalOutput").ap()

    with tile.TileContext(nc) as tc, ExitStack() as ctx:
        singles = ctx.enter_context(tc.tile_pool(name="singles", bufs=1))
        wpool = ctx.enter_context(tc.tile_pool(name="wpool", bufs=2))
        persist = ctx.enter_context(tc.tile_pool(name="persist", bufs=1))
        xpool = ctx.enter_context(tc.tile_pool(name="xpool", bufs=1))
        # PSUM (8 banks): spool 3 (proj acc / scores / rb / out-proj acc)
        # + sw 1 + pa 4
        spool = ctx.enter_context(tc.tile_pool(name="spool", bufs=3, space="PSUM"))
        papool = ctx.enter_context(tc.tile_pool(name="papool", bufs=2, space="PSUM"))

        qr_sb = persist.tile([128, 8, TOK], bf16, tag="qr")
        kr_sb = persist.tile([128, 8, TOK], bf16, tag="kr")
        # v with a ones column appended per head: [kpos, tile, head, 65]
        va_sb = persist.tile([128, NT, HEADS, HDIM + 1], bf16, tag="va")
        nc.vector.memset(va_sb[:, :, :, HDIM:HDIM + 1], 1.0)
        wo_sb = persist.tile([128, 8, EMBED], bf16, tag="wo")

        def load_w(dram, w=None):
            if w is None:
                w = wpool.tile([128, 8, EMBED], bf16, tag="w")
            r = dram.rearrange("(a p) n -> p a n", p=128)
            for a in range(8):
                nc.sync.dma_start(out=w[:, a, :], in_=r[:, a, :])
            return w

        # issue DMAs in consumption order: x/wq interleaved first so the
        # first matmul group starts ~2us in, bulky late-use loads last
        x_sb = xpool.tile([128, 8, TOK], bf16, tag="x")
        xrr = xT_d.rearrange("(a p) t -> p a t", p=128)
        wq = wpool.tile([128, 8, EMBED], bf16, tag="w")
        wqr = wq_d.rearrange("(a p) n -> p a n", p=128)
        for a in range(8):
            nc.sync.dma_start(out=x_sb[:, a, :], in_=xrr[:, a, :])
            nc.sync.dma_start(out=wq[:, a, :], in_=wqr[:, a, :])
        perm_sb = singles.tile([128, 128], bf16, tag="permM")
        nc.sync.dma_start(out=perm_sb, in_=perm_d)
        qb_sb = singles.tile([128, 8], f32, tag="qb")
        nc.sync.dma_start(out=qb_sb, in_=qb_d)
        qbr_sb = singles.tile([128, 8], f32, tag="qbr")
        nc.sync.dma_start(out=qbr_sb, in_=qbr_d)

        # ----------------------------------------------- projections + RoPE
        with tc.tile_pool(name="cspool", bufs=1) as cspool, \
             tc.tile_pool(name="swpool", bufs=1, space="PSUM") as swpool, \
             tc.tile_pool(name="rope", bufs=2) as rope:
            cos_sb = cspool.tile([128, TOK], bf16, tag="cos")
            nc.sync.dma_start(out=cos_sb, in_=cosT_d)
            sin_sb = cspool.tile([128, TOK], bf16, tag="sin")
            nc.sync.dma_start(out=sin_sb, in_=sinT_d)
            sel0_sb = singles.tile([1, 128], DT.float32r, tag="sel0")
            nc.sync.dma_start(out=sel0_sb, in_=sel_d[0:1, :])
            sel1_sb = singles.tile([1, 128], DT.float32r, tag="sel1")
            nc.sync.dma_start(out=sel1_sb, in_=sel_d[1:2, :])
            sel_sb = (sel0_sb, sel1_sb)
            mask_sb = singles.tile([128, mask_cols], bf16, tag="maskb")
            nc.sync.dma_start(out=mask_sb, in_=mask_d)

            def rope_proj(w_sb, with_bias, out_sb):
                for m in range(8):
                    for c3 in range(3):
                        t5 = bass.ts(c3, 512)
                        ps = spool.tile([128, 512], f32, tag="s")
                        for a in range(8):
                            nc.tensor.matmul(ps, w_sb[:, a, bass.ts(m, 128)],
                                             x_sb[:, a, t5], start=(a == 0),
                                             stop=(a == 7))
                        qc = rope.tile([128, 512], bf16, tag="qc")
                        nc.scalar.copy(qc, ps)
                        # rotate_half via PE permutation matmul (keeps the
                        # Sync queue free and the PE stream dense)
                        swp = swpool.tile([128, 512], f32, tag="sw")
                        nc.tensor.matmul(swp, perm_sb, qc, start=True,
                                         stop=True)
                        sw = rope.tile([128, 512], bf16, tag="sws")
                        nc.scalar.copy(sw, swp)
                        m1 = rope.tile([128, 512], bf16, tag="m1")
                        m2 = rope.tile([128, 512], bf16, tag="m2")
                        if with_bias:
                            nc.vector.scalar_tensor_tensor(
                                out=m1, in0=qc, scalar=qb_sb[:, m:m + 1],
                                in1=cos_sb[:, t5], op0=ADD, op1=MUL)
                            nc.vector.scalar_tensor_tensor(
                                out=m2, in0=sw, scalar=qbr_sb[:, m:m + 1],
                                in1=sin_sb[:, t5], op0=ADD, op1=MUL)
                        else:
                            nc.vector.tensor_mul(m1, qc, cos_sb[:, t5])
                            nc.vector.tensor_mul(m2, sw, sin_sb[:, t5])
                        nc.vector.tensor_add(out_sb[:, m, t5], m1, m2)

            rope_proj(wq, True, qr_sb)
            wk = load_w(wk_d)
            rope_proj(wk, False, kr_sb)
            wv = load_w(wv_d)
            load_w(wo_d, w=wo_sb)

        def v_group(tt):
            for n2 in range(2):
                ps = spool.tile([128, 512], f32, tag="s")
                for a in range(8):
                    nc.tensor.matmul(ps, x_sb[:, a, bass.ts(tt, 128)],
                                     wv[:, a, bass.ts(n2, 512)],
                                     start=(a == 0), stop=(a == 7))
                nc.scalar.copy(va_sb[:, tt, bass.ts(n2, 8), 0:HDIM],
                               ps.rearrange("p (h d) -> p h d", d=HDIM))

        # v tiles 0-3 are needed by cch0's PV; the rest are interleaved into
        # cch0's head loop so the PE has work while Act runs exps
        for tt in range(4):
            v_group(tt)

        # ------------------------------------------------------- attention --
        with tc.tile_pool(name="epool", bufs=1) as epool, \
             tc.tile_pool(name="dpool", bufs=2) as dpool, \
             tc.tile_pool(name="attnp", bufs=2) as attnp, \
             tc.tile_pool(name="rbpool", bufs=1, space="PSUM") as rbpool, \
             tc.tile_pool(name="ypool", bufs=2) as ypool:
            def outproj_m(attn_prev, cchp, m):
                py = spool.tile([128, 512], f32, tag="s")
                for r in range(8):
                    nc.tensor.matmul(py, wo_sb[:, r, bass.ts(m, 128)],
                                     attn_prev[:, r, :], start=(r == 0),
                                     stop=(r == 7))
                ys = ypool.tile([128, 512], bf16, tag="ys")
                nc.vector.tensor_copy(ys, py)
                nc.sync.dma_start(
                    out=yT_d[bass.ts(m, 128), bass.ts(cchp, 512)], in_=ys)

            attn_prev = None
            for cch in range(NQC):
                q0 = cch * 512
                klist = structure[cch]
                attn_sb = attnp.tile([128, 8, 512], bf16, tag="attn")
                pending = None  # (hp, pas, rcps) awaiting rb+normalize
                for hp in range(8):
                    # dependency-free dense PE work interleaved per head
                    # pair (v-proj tiles 4-7 in cch0, tiles 8-11 plus the
                    # previous out-projection in cch1, out-projection in
                    # cch2) keeps the HAM clock gate warm while Act runs
                    # the exp stream
                    if cch == 0:
                        if hp < 4:
                            v_group(4 + hp)
                    else:
                        outproj_m(attn_prev, cch - 1, hp)
                        if cch == 1 and hp < 4:
                            v_group(8 + hp)
                    pas = [papool.tile([HDIM + 1, 512], f32, tag=f"pa{i}",
                                       name=f"pa{i}")
                           for i in range(2)]
                    # per chunk: QK -> exp -> mask -> PV, pipelined by the
                    # 3-deep score pool; e tiles rotate 4 tags per head
                    for i in range(2):
                        h = 2 * hp + i
                        for ik, (kj, qlo, qhi, runs) in enumerate(klist):
                            nq = (qhi - qlo + 1) * 128
                            krs = kr_sb[bass.ds((h % 2) * 64, 64), h // 2,
                                        bass.ts(kj, 128)]
                            qrs = qr_sb[bass.ds((h % 2) * 64, 64), h // 2,
                                        bass.ds(qlo * 128, nq)]
                            ps = spool.tile([128, 512], f32, tag="s")
                            nc.tensor.matmul(ps[:, 0:nq], krs, qrs,
                                             start=True, stop=True)
                            e = epool.tile([128, 512], bf16,
                                           tag=f"e{i}_{ik % 4}")
                            nc.scalar.activation(e[:, 0:nq], ps[:, 0:nq], EXP,
                                                 scale=0.125)
                            for (qa, qb, mo) in runs:
                                w = (qb - qa) * 128
                                off = (qa - qlo) * 128
                                nc.vector.tensor_mul(
                                    e[:, bass.ds(off, w)],
                                    e[:, bass.ds(off, w)],
                                    mask_sb[:, bass.ds(mo, w)])
                            qoff = qlo * 128 - q0
                            nc.tensor.matmul(
                                pas[i][:, bass.ds(qoff, nq)],
                                va_sb[:, kj, h, :], e[:, 0:nq],
                                start=(ik == 0), stop=(ik == len(klist) - 1),
                                skip_group_check=True)
                    # denominators -> fast reciprocal (via SBUF; the custom
                    # DVE op reads garbage from PSUM) -> f32r for the
                    # 1-cycle/row broadcast matmul
                    rcps = []
                    for i in range(2):
                        den = dpool.tile([1, 512], f32, tag=f"den{i}")
                        nc.vector.tensor_copy(den, pas[i][HDIM:HDIM + 1, :])
                        rcp = dpool.tile([1, 512], f32, tag=f"rcp{i}")
                        nc.vector.reciprocal_approx_fast(out=rcp, in_=den)
                        rcpr = dpool.tile([1, 512], DT.float32r, tag=f"rcpr{i}")
                        nc.vector.tensor_scalar_mul(rcpr, rcp, 1.0)
                        rcps.append(rcpr)
                    if pending is not None:
                        _flush_norm(nc, rbpool, dpool, attn_sb, sel_sb,
                                    pending)
                    pending = (hp, pas, rcps)
                _flush_norm(nc, rbpool, dpool, attn_sb, sel_sb, pending)
                attn_prev = attn_sb

            # ------------------------------------ final cch out-projection
            for m in range(8):
                outproj_m(attn_prev, NQC - 1, m)
    nc.compile()
    return nc


def _flush_norm(nc, rbpool, dpool, attn_sb, sel_sb, pending):
    """rb = broadcast(rcp) over partitions via two contract-1 matmuls, then
    normalize both heads of the pair straight out of PSUM."""
    hp, pas, rcps = pending
    f32 = DT.float32
    bf16 = DT.bfloat16
    rb = rbpool.tile([128, 512], f32, tag="rb")
    nc.tensor.matmul(rb, sel_sb[0], rcps[0], start=True, stop=False)
    nc.tensor.matmul(rb, sel_sb[1], rcps[1], start=False, stop=True)
    rbs = dpool.tile([128, 512], bf16, tag="rbs")
    nc.scalar.copy(rbs, rb)
    for i in range(2):
        h = 2 * hp + i
        nc.vector.tensor_mul(
            attn_sb[bass.ds((h % 2) * 64, 64), h // 2, :],
            pas[i][0:HDIM, :], rbs[bass.ds(i * 64, 64), :])


# ------------------------------------------------------------- host driver --

def _host_prep(hidden, cos, sin, seq_lens, core_chunks, structure, mask_cols):
    starts = np.concatenate([[0], np.cumsum(seq_lens)]).astype(np.int64)
    per_core = []
    sgn = np.concatenate([-np.ones(32, F32), np.ones(32, F32)])
    for c in range(NCORES):
        tokmap = np.full(TOK, -1, np.int64)
        segid = np.full(TOK, -1, np.int64)
        pos = np.zeros(TOK, np.int64)
        for (s, t0, L) in core_chunks[c]:
            sl = slice(t0 * 128, t0 * 128 + L)
            tokmap[sl] = np.arange(starts[s], starts[s] + L)
            segid[sl] = s
            pos[sl] = np.arange(L)
        real = tokmap >= 0
        x = np.zeros((TOK, EMBED), F32)
        x[real] = hidden[tokmap[real]]
        cs = np.zeros((TOK, HDIM), F32)
        sn = np.zeros((TOK, HDIM), F32)
        cs[real] = cos[tokmap[real]]
        sn[real] = sin[tokmap[real]]
        cosT = np.tile(np.ascontiguousarray(cs.T), (2, 1)).astype(BF16)
        sinT = np.tile(np.ascontiguousarray(sn.T) * sgn[:, None],
                       (2, 1)).astype(BF16)
        # packed mask tiles for the masked runs only
        maskb = np.zeros((128, mask_cols), BF16)
        for cch in range(NQC):
            for (kj, qlo, qhi, runs) in structure[cch]:
                ks = slice(kj * 128, kj * 128 + 128)
                segk, posk = segid[ks], pos[ks]
                for (qa, qb, mo) in runs:
                    qs = slice(qa * 128, qb * 128)
                    segq, posq = segid[qs], pos[qs]
                    ok = ((segk[:, None] == segq[None, :])
                          & (segk[:, None] >= 0)
                          & (posk[:, None] <= posq[None, :]))
                    # force diag to 1 so padded q columns can't hit 0/0
                    gk = np.arange(kj * 128, kj * 128 + 128)
                    gq = np.arange(qa * 128, qb * 128)
                    ok |= (gk[:, None] == gq[None, :])
                    maskb[:, mo:mo + (qb - qa) * 128] = ok.astype(BF16)
        per_core.append(dict(tokmap=tokmap,
                             xT=np.ascontiguousarray(x.T).astype(BF16),
                             cosT=cosT, sinT=sinT, maskb=maskb))
    return per_core


def kernel(hidden_states, cos, sin, q_w, q_b, k_w, v_w, v_b, out_w, out_b,
           seq_len, max_seqlen):
    global LAST_EXEC_NS
    hidden = np.asarray(hidden_states, F32)
    cos = np.asarray(cos, F32)
    sin = np.asarray(sin, F32)
    seq_lens = [int(v) for v in np.asarray(seq_len)]

    core_chunks, structure, mask_cols = _build_plan(seq_lens)
    key = (structure, mask_cols)
    if key not in _CACHE:
        _CACHE[key] = _emit_program(structure, mask_cols)
    nc = _CACHE[key]

    per_core = _host_prep(hidden, cos, sin, seq_lens, core_chunks, structure,
                          mask_cols)
    sel2 = np.zeros((2, 128), F32)
    sel2[0, 0:64] = 1.0
    sel2[1, 64:128] = 1.0
    qb = np.asarray(q_b, F32).reshape(8, 128).T.copy()  # [128, 8]
    # rotate_half permutation of the bias (sign lives in sinT)
    qbr_full = np.asarray(q_b, F32).reshape(EMBED)
    perm = np.arange(EMBED).reshape(-1, 64)
    perm = np.concatenate([perm[:, 32:], perm[:, :32]], axis=1).reshape(-1)
    qbr = qbr_full[perm].reshape(8, 128).T.copy()
    permM = np.zeros((128, 128), F32)
    for i in range(128):
        src = (i // 64) * 64 + ((i % 64) + 32) % 64
        permM[src, i] = 1.0
    shared = {
        "wqT": np.ascontiguousarray(np.asarray(q_w, F32).T).astype(BF16),
        "wkT": np.ascontiguousarray(np.asarray(k_w, F32).T).astype(BF16),
        "wvT": np.ascontiguousarray(np.asarray(v_w, F32).T).astype(BF16),
        "woT": np.ascontiguousarray(np.asarray(out_w, F32).T).astype(BF16),
        "qb": qb, "qbr": qbr,
        "sel2": sel2,
        "permM": permM.astype(BF16),
    }
    in_maps = []
    for c in range(NCORES):
        pc = per_core[c]
        in_maps.append({**shared, "xT": pc["xT"], "cosT": pc["cosT"],
                        "sinT": pc["sinT"], "maskb": pc["maskb"]})

    trace = os.environ.get("BASS_KERNEL_TRACE", "0") == "1"
    if trace:
        _install_ntff_shim()
    import time as _time
    _t0 = _time.time()
    res = run_bass_kernel_spmd(nc, in_maps, core_ids=list(range(NCORES)),
                               trace=trace)
    LAST_EXEC_NS = res.exec_time_ns
    globals()["LAST_TRACE"] = res.instructions_and_trace
    globals()["LAST_RUN_WALL_S"] = _time.time() - _t0

    # v-bias and out-bias folded here: y = y_dev + (v_b @ out_w.T + out_b)
    ob2 = (np.asarray(v_b, F32) @ np.asarray(out_w, F32).T
           + np.asarray(out_b, F32))
    T = hidden.shape[0]
    out = np.zeros((T, EMBED), F32)
    for c in range(NCORES):
        tokmap = per_core[c]["tokmap"]
        real = tokmap >= 0
        yT = np.asarray(res.results[c]["yT"], F32)
        out[tokmap[real]] = yT.T[real] + ob2[None, :]
    return out
